# revision 11
# baseline (speedup 1.0000x reference)
"""ConsMax attention kernel for Trainium2, sharded over 8 NeuronCores.

Sharding: 2 batches x 4 head-groups (4 heads each) = 8 cores.
Each core computes its batch's q/k/v for its 4 heads, full attention over
S=2048, and a partial output projection; the host sums the 4 head-group
partials per batch (the tensor-parallel reduce) and adds bo.

ConsMax math: probs = exp(scores - beta - rowmax(scores - beta)) / gamma
            = exp(scores - rowmax(scores)) / gamma        (beta cancels)
gamma is folded into Wo on the host. The rowmax subtraction commutes
through the PV matmul: ctx = (exp(scores) @ v) / max(exp(scores)) applied
as a per-query-column rescale of ctx^T, using max(exp(s)) = exp(max(s))
(monotonicity). The max is taken over the exp'd probability tiles (pu)
with a bf16 tensor_tensor(max) tree over key chunks + a PE transpose +
free-dim reduce, so no separate scores pass is needed. exp(scores) cannot
overflow here: |q.k|/8 stays O(1) for this problem's 0.02-scaled weights.

Device layouts (per core):
  qT,kT  [256, 2048] fp32  (d on partitions; pair chunk p holds heads 2p,2p+1)
  v      [2048, 256] bf16  (ks on partitions)
  pu     exp'd scores, transposed [ks, qs], bf16
  ctxT   [256, 2048] fp32
"""

import os
import numpy as np

import concourse.bacc as bacc
import concourse.bass as bass
import concourse.tile as tile
from concourse import mybir
from concourse.bass import ts, ds
from concourse.bass_utils import run_bass_kernel_spmd
from concourse.masks import make_identity

B, S, HID, NH, HD = 2, 2048, 1024, 16, 64
NCORES = 8
NGROUPS = 4          # head groups (cores per batch)
GH = NH // NGROUPS   # heads per group = 4
C = GH * HD          # head-group dim = 256
P = 128
FP32 = mybir.dt.float32
BF16 = mybir.dt.bfloat16

_last_results = None
_cached = None


def _build_program():
    nc = bacc.Bacc(
        "TRN2", target_bir_lowering=False, debug=False, num_devices=NCORES
    )

    xT_d = nc.dram_tensor("xT", [HID, S], FP32, kind="ExternalInput").ap()
    wq_d = nc.dram_tensor("wqT", [HID, C], FP32, kind="ExternalInput").ap()
    wk_d = nc.dram_tensor("wkT", [HID, C], FP32, kind="ExternalInput").ap()
    wv_d = nc.dram_tensor("wvT", [HID, C], FP32, kind="ExternalInput").ap()
    wo_d = nc.dram_tensor("woT", [C, HID], FP32, kind="ExternalInput").ap()
    bq_d = nc.dram_tensor("bq", [1, C], FP32, kind="ExternalInput").ap()
    bk_d = nc.dram_tensor("bk", [1, C], FP32, kind="ExternalInput").ap()
    bv_d = nc.dram_tensor("bv", [1, C], FP32, kind="ExternalInput").ap()
    mb_d = nc.dram_tensor("mb", [P, S // P], FP32, kind="ExternalInput").ap()
    sel_d = nc.dram_tensor("sel", [16, 8, P], FP32, kind="ExternalInput").ap()
    out_d = nc.dram_tensor("outp", [S, HID], FP32, kind="ExternalOutput").ap()

    HC = HID // P        # 8 hidden chunks
    SC = S // P          # 16 seq chunks
    NB = S // 512        # 4 n-blocks of 512
    NQ = 2               # qs super-blocks
    QW = S // NQ         # 1024

    with tile.TileContext(nc) as tc:
        with (
            tc.tile_pool(name="const", bufs=1) as const,
            tc.tile_pool(name="persist", bufs=1) as persist,
            tc.tile_pool(name="work", bufs=1) as work,
        ):
            # ---- constants ----
            ident = const.tile([P, P], FP32)
            make_identity(nc, ident)
            ones_s = const.tile([1, 512], FP32)
            nc.vector.memset(ones_s, 1.0)
            # fbcast selection weights (host-built): sel16[k, qbl, r]
            # = 1 iff k == 2*qbl + (r >= 64)
            sel16 = const.tile([16, 8, P], FP32)
            nc.sync.dma_start(sel16[:], sel_d[:])
            ident_bf = const.tile([P, P], BF16)
            make_identity(nc, ident_bf)
            mb_s = const.tile([P, SC], FP32)
            nc.sync.dma_start(mb_s[:], mb_d[:])
            bq_s = const.tile([1, C], FP32)
            nc.sync.dma_start(bq_s[:], bq_d[:])
            bk_s = const.tile([1, C], FP32)
            nc.sync.dma_start(bk_s[:], bk_d[:])
            bv_s = const.tile([1, C], FP32)
            nc.sync.dma_start(bv_s[:], bv_d[:])
            wo_s = const.tile([P, 2, HID], FP32)
            nc.sync.dma_start(wo_s[:], wo_d.rearrange("(a p) o -> p a o", p=P))

            # ---- persistent activations ----
            qT = persist.tile([P, 2, S], BF16)    # [d, pair, qs]
            kT = persist.tile([P, 2, S], BF16)
            vv = persist.tile([P, SC, C], BF16)   # [ks, kchunk, c]
            ctxT = persist.tile([P, 2, S], FP32)  # [c, pair, qs]
            mcols = persist.tile([P, 2, SC, 2], FP32)  # max(pu), (pair, qb, l)

            # ================= P0: projections =================
            with (
                tc.tile_pool(name="pj", bufs=4, space="PSUM") as pj,
                tc.tile_pool(name="xw_pool", bufs=1) as xw_pool,
            ):
                xTs = xw_pool.tile([P, HC, S], FP32)
                nc.sync.dma_start(xTs[:], xT_d.rearrange("(a p) s -> p a s", p=P))
                wq_s = xw_pool.tile([P, HC, C], FP32)
                nc.sync.dma_start(wq_s[:], wq_d.rearrange("(a p) c -> p a c", p=P))
                wk_s = xw_pool.tile([P, HC, C], FP32)
                nc.sync.dma_start(wk_s[:], wk_d.rearrange("(a p) c -> p a c", p=P))
                wv_s = xw_pool.tile([P, HC, C], FP32)
                nc.sync.dma_start(wv_s[:], wv_d.rearrange("(a p) c -> p a c", p=P))

                # qT / kT: [256, S] = W_g @ x^T  (+ bias via K=1 matmul)
                for w_s, b_s, dst in ((wq_s, bq_s, qT), (wk_s, bk_s, kT)):
                    for m in range(2):
                        for nb in range(NB):
                            ps = pj.tile([P, 512], FP32, tag="pj")
                            for h in range(HC):
                                nc.tensor.matmul(
                                    ps,
                                    lhsT=w_s[:, h, ts(m, P)],
                                    rhs=xTs[:, h, ts(nb, 512)],
                                    start=(h == 0),
                                    stop=False,
                                )
                            nc.tensor.matmul(
                                ps,
                                lhsT=b_s[:, ts(m, P)],
                                rhs=ones_s[:, 0:512],
                                start=False,
                                stop=True,
                            )
                            nc.vector.tensor_copy(out=dst[:, m, ts(nb, 512)], in_=ps)

                # v: [S, 256] = x @ Wv_g^T (+ bias), bf16
                for sc in range(SC):
                    ps = pj.tile([P, 512], FP32, tag="pj")
                    pv = ps[:, :C]
                    for h in range(HC):
                        nc.tensor.matmul(
                            pv,
                            lhsT=xTs[:, h, ts(sc, P)],
                            rhs=wv_s[:, h, :],
                            start=(h == 0),
                            stop=False,
                        )
                    nc.tensor.matmul(
                        pv,
                        lhsT=ones_s[:, 0:P],
                        rhs=bv_s[:],
                        start=False,
                        stop=True,
                    )
                    nc.vector.tensor_copy(out=vv[:, sc, :], in_=pv)

            # ============ attention, per qs super-block ============
            with (
                # sT/fbcast psum tiles in one 2-slot tag (4 banks);
                # ctxT accumulators, P4 out tiles and max-transposes in
                # another (4 banks).
                tc.tile_pool(name="stp", bufs=2, space="PSUM") as stp,
                tc.tile_pool(name="accp", bufs=2, space="PSUM") as accp,
                tc.tile_pool(name="pu_pool", bufs=32) as pu_pool,
                tc.tile_pool(name="fb_pool", bufs=2) as fb_pool,
                tc.tile_pool(name="osb_pool", bufs=3) as osb_pool,
                tc.tile_pool(name="frp_pool", bufs=2) as frp_pool,
            ):
                for Q in range(NQ):
                    for p in range(2):
                        # ---- P2: sT scores (transposed) -> pu = exp ----
                        pu_tiles = [[None] * SC, [None] * SC]
                        for c in range(SC):
                            for l in range(2):
                                rows = slice(64 * l, 64 * l + 64)
                                st = stp.tile([P, QW], FP32, tag="B")
                                for u in range(2):
                                    nc.tensor.matmul(
                                        st[:, ts(u, 512)],
                                        lhsT=kT[rows, p, ts(c, P)],
                                        rhs=qT[rows, p, ds(Q * QW + u * 512, 512)],
                                        start=True,
                                        stop=True,
                                    )
                                pu = pu_pool.tile([P, QW], BF16, tag="pu")
                                nc.scalar.activation(
                                    out=pu,
                                    in_=st,
                                    func=mybir.ActivationFunctionType.Exp,
                                    bias=mb_s[:, c : c + 1],
                                    scale=0.125,
                                )
                                pu_tiles[l][c] = pu

                        # ---- P3 part 1: PV matmuls into ctx psum ----
                        cx = accp.tile([P, QW], FP32, tag="C")
                        for c in range(SC):
                            for l in range(2):
                                for u in range(2):
                                    nc.tensor.matmul(
                                        cx[ds(64 * l, 64), ts(u, 512)],
                                        lhsT=vv[:, c, ds(128 * p + 64 * l, 64)],
                                        rhs=pu_tiles[l][c][:, ts(u, 512)],
                                        start=(c == 0),
                                        stop=(c == SC - 1),
                                    )

                        # ---- rowmax(pu): in-place chunk-pair max tree over
                        # the pu tiles (safe: PV has consumed them), then a
                        # PE transpose per query block + free-dim reduce ----
                        for l in range(2):
                            stride = 1
                            while stride < SC:
                                for i in range(0, SC, 2 * stride):
                                    nc.vector.tensor_tensor(
                                        out=pu_tiles[l][i][:],
                                        in0=pu_tiles[l][i][:],
                                        in1=pu_tiles[l][i + stride][:],
                                        op=mybir.AluOpType.max,
                                    )
                                stride *= 2
                            R = pu_tiles[l][0]
                            for b8 in range(8):
                                mtp = accp.tile([P, P], BF16, tag="C")
                                nc.tensor.transpose(mtp, R[:, ts(b8, P)], ident_bf)
                                nc.vector.reduce_max(
                                    out=mcols[:, p, Q * 8 + b8, l : l + 1],
                                    in_=mtp,
                                    axis=mybir.AxisListType.X,
                                )

                        # ---- frTp = 1/max(pu) transposed to qs-free ----
                        mt = accp.tile([16, P], FP32, tag="C")
                        nc.tensor.transpose(
                            mt,
                            mcols[:, p, ds(Q * 8, 8), :].rearrange("p a b -> p (a b)"),
                            ident,
                        )
                        frTp = frp_pool.tile([16, P], FP32, tag="fr")
                        nc.vector.reciprocal(out=frTp, in_=mt)

                        # ---- fbcast: broadcast frTp to [128, QW] columns ----
                        fb_ps = stp.tile([P, QW], FP32, tag="B")
                        for qbl in range(8):
                            nc.tensor.matmul(
                                fb_ps[:, ts(qbl, P)],
                                lhsT=sel16[:, qbl, :],
                                rhs=frTp[:],
                                start=True,
                                stop=True,
                            )
                        fb_sb = fb_pool.tile([P, QW], FP32, tag="fb")
                        nc.vector.tensor_copy(out=fb_sb, in_=fb_ps)

                        # ---- P3 part 2: rescale ctx by 1/max and store ----
                        nc.vector.tensor_tensor(
                            out=ctxT[:, p, ds(Q * QW, QW)],
                            in0=cx[:],
                            in1=fb_sb[:],
                            op=mybir.AluOpType.mult,
                        )


                    # ---- P4: output projection for this Q ----
                    for qb in range(Q * 8, Q * 8 + 8):
                        op_ps = accp.tile([P, 1024], FP32, tag="C")
                        for ob in range(2):
                            for p in range(2):
                                nc.tensor.matmul(
                                    op_ps[:, ts(ob, 512)],
                                    lhsT=ctxT[:, p, ts(qb, P)],
                                    rhs=wo_s[:, p, ds(ob * 512, 512)],
                                    start=(p == 0),
                                    stop=(p == 1),
                                )
                        o_sb = osb_pool.tile([P, 1024], FP32, tag="osb")
                        nc.vector.tensor_copy(out=o_sb, in_=op_ps)
                        nc.sync.dma_start(out_d[ts(qb, P), :], o_sb)

    nc.compile()
    return nc


def _sel_const():
    sel = np.zeros((16, 8, P), dtype=np.float32)
    for qbl in range(8):
        sel[2 * qbl, qbl, 0:64] = 1.0
        sel[2 * qbl + 1, qbl, 64:128] = 1.0
    return sel


def _prep_inputs(hidden_states, attention_mask, Wq, bq, Wk, bk, Wv, bv,
                 Wo, bo, beta, gamma):
    g_scalar = float(np.asarray(gamma).reshape(-1)[0])
    in_maps = []
    for core in range(NCORES):
        b, g = core // NGROUPS, core % NGROUPS
        sl = slice(g * C, (g + 1) * C)
        mb = ((1.0 - np.asarray(attention_mask)[b]) * -10000.0).astype(np.float32)
        in_maps.append({
            "xT": np.ascontiguousarray(np.asarray(hidden_states)[b].T,
                                       dtype=np.float32),
            "wqT": np.ascontiguousarray(np.asarray(Wq)[sl, :].T, dtype=np.float32),
            "wkT": np.ascontiguousarray(np.asarray(Wk)[sl, :].T, dtype=np.float32),
            "wvT": np.ascontiguousarray(np.asarray(Wv)[sl, :].T, dtype=np.float32),
            "woT": (np.ascontiguousarray(np.asarray(Wo)[:, sl].T, dtype=np.float32)
                    / g_scalar),
            "bq": np.asarray(bq)[sl].reshape(1, C).astype(np.float32),
            "bk": np.asarray(bk)[sl].reshape(1, C).astype(np.float32),
            "bv": np.asarray(bv)[sl].reshape(1, C).astype(np.float32),
            "mb": np.ascontiguousarray(mb.reshape(S // P, P).T),
            "sel": _sel_const(),
        })
    return in_maps


def kernel(**inputs):
    global _cached, _last_results
    if _cached is None:
        _cached = _build_program()
    nc = _cached
    in_maps = _prep_inputs(**inputs)
    os.environ["BASS_NEVER_TRACE"] = "1"  # no NTFF hook on this axon client
    res = run_bass_kernel_spmd(nc, in_maps, core_ids=list(range(NCORES)))
    _last_results = res
    bo = np.asarray(inputs["bo"], dtype=np.float32)
    out = np.zeros((B, S, HID), dtype=np.float32)
    for core in range(NCORES):
        out[core // NGROUPS] += res.results[core]["outp"]
    out += bo[None, None, :]
    return out


# revision 14
# speedup vs baseline: 2.5753x; 2.5753x over previous
"""ConsMax attention kernel for Trainium2, sharded over 8 NeuronCores.

Sharding: 2 batches x 4 head-groups (4 heads each) = 8 cores.
Each core computes its batch's q/k/v for its 4 heads, full attention over
S=2048, and a partial output projection; the host sums the 4 head-group
partials per batch (the tensor-parallel reduce) and adds bo.

ConsMax math: probs = exp(scores - beta - rowmax(scores - beta)) / gamma
            = exp(scores - rowmax(scores)) / gamma        (beta cancels)
gamma is folded into Wo on the host. The rowmax subtraction commutes
through the PV matmul: ctx = (exp(scores) @ v) / max(exp(scores)) applied
as a per-query-column rescale of ctx^T, using max(exp(s)) = exp(max(s))
(monotonicity). The max is taken over the exp'd probability tiles (pu)
with a bf16 tensor_tensor(max) tree over key chunks + a PE transpose +
free-dim reduce, so no separate scores pass is needed. exp(scores) cannot
overflow here: |q.k|/8 stays O(1) for this problem's 0.02-scaled weights.

Device layouts (per core):
  qT,kT  [256, 2048] fp32  (d on partitions; pair chunk p holds heads 2p,2p+1)
  v      [2048, 256] bf16  (ks on partitions)
  pu     exp'd scores, transposed [ks, qs], bf16
  ctxT   [256, 2048] fp32
"""

import os
import ml_dtypes
import numpy as np

import concourse.bacc as bacc
import concourse.bass as bass
import concourse.tile as tile
from concourse import mybir
from concourse.bass import ts, ds
from concourse.bass_utils import run_bass_kernel_spmd
from concourse.masks import make_identity

B, S, HID, NH, HD = 2, 2048, 1024, 16, 64
NCORES = 8
NGROUPS = 4          # head groups (cores per batch)
GH = NH // NGROUPS   # heads per group = 4
C = GH * HD          # head-group dim = 256
P = 128
FP32 = mybir.dt.float32
BF16 = mybir.dt.bfloat16

_last_results = None
_cached = None


def _build_program():
    nc = bacc.Bacc(
        "TRN2", target_bir_lowering=False, debug=False, num_devices=NCORES,
        num_swdge_queues=4,
    )

    xT_d = nc.dram_tensor("xT", [HID, S], BF16, kind="ExternalInput").ap()
    wq_d = nc.dram_tensor("wqT", [HID, C], BF16, kind="ExternalInput").ap()
    wk_d = nc.dram_tensor("wkT", [HID, C], BF16, kind="ExternalInput").ap()
    wv_d = nc.dram_tensor("wvT", [HID, C], BF16, kind="ExternalInput").ap()
    wo_d = nc.dram_tensor("woT", [C, HID], BF16, kind="ExternalInput").ap()
    bq_d = nc.dram_tensor("bq", [1, C], BF16, kind="ExternalInput").ap()
    bk_d = nc.dram_tensor("bk", [1, C], BF16, kind="ExternalInput").ap()
    bv_d = nc.dram_tensor("bv", [1, C], BF16, kind="ExternalInput").ap()
    mb_d = nc.dram_tensor("mb", [P, S // P], FP32, kind="ExternalInput").ap()
    sel_d = nc.dram_tensor("sel", [16, 8, P], FP32, kind="ExternalInput").ap()
    out_d = nc.dram_tensor("outp", [S, HID], FP32, kind="ExternalOutput").ap()

    HC = HID // P        # 8 hidden chunks
    SC = S // P          # 16 seq chunks
    NB = S // 512        # 4 n-blocks of 512
    NQ = 2               # qs super-blocks
    QW = S // NQ         # 1024

    with tile.TileContext(nc) as tc:
        with (
            tc.tile_pool(name="const", bufs=1) as const,
            tc.tile_pool(name="persist", bufs=1) as persist,
            tc.tile_pool(name="work", bufs=1) as work,
        ):
            # ---- constants ----
            ident = const.tile([P, P], FP32)
            make_identity(nc, ident)
            ones_s = const.tile([1, 512], BF16)
            nc.vector.memset(ones_s, 1.0)
            # fbcast selection weights (host-built): sel16[k, qbl, r]
            # = 1 iff k == 2*qbl + (r >= 64)
            sel16 = const.tile([16, 8, P], FP32)
            nc.sync.dma_start(sel16[:], sel_d[:])
            ident_bf = const.tile([P, P], BF16)
            make_identity(nc, ident_bf)
            mb_s = const.tile([P, SC], FP32)
            nc.sync.dma_start(mb_s[:], mb_d[:])
            bq_s = const.tile([1, C], BF16)
            nc.sync.dma_start(bq_s[:], bq_d[:])
            bk_s = const.tile([1, C], BF16)
            nc.sync.dma_start(bk_s[:], bk_d[:])
            bv_s = const.tile([1, C], BF16)
            nc.sync.dma_start(bv_s[:], bv_d[:])
            wo_s = const.tile([P, 2, HID], BF16)
            nc.sync.dma_start(wo_s[:], wo_d.rearrange("(a p) o -> p a o", p=P))

            # ---- persistent activations ----
            qT = persist.tile([P, 2, S], BF16)    # [d, pair, qs]
            kT = persist.tile([P, 2, S], BF16)
            vv = persist.tile([P, SC, C], BF16)   # [ks, kchunk, c]
            ctxT = persist.tile([P, 2, S], BF16)  # [c, pair, qs]
            mcols = persist.tile([P, 2, SC, 2], FP32)  # max(pu), (pair, qb, l)

            # ================= P0: projections =================
            with (
                tc.tile_pool(name="pj", bufs=4, space="PSUM") as pj,
                tc.tile_pool(name="xw_pool", bufs=1) as xw_pool,
            ):
                xTs = xw_pool.tile([P, HC, S], BF16)
                xr = xT_d.rearrange("(a p) s -> p a s", p=P)
                for h in range(HC):
                    nc.sync.dma_start(xTs[:, h], xr[:, h])
                wq_s = xw_pool.tile([P, HC, C], BF16)
                nc.sync.dma_start(wq_s[:], wq_d.rearrange("(a p) c -> p a c", p=P))
                wk_s = xw_pool.tile([P, HC, C], BF16)
                nc.sync.dma_start(wk_s[:], wk_d.rearrange("(a p) c -> p a c", p=P))
                wv_s = xw_pool.tile([P, HC, C], BF16)
                nc.sync.dma_start(wv_s[:], wv_d.rearrange("(a p) c -> p a c", p=P))

                # qT / kT: [256, S] = W_g @ x^T  (+ bias via K=1 matmul)
                for w_s, b_s, dst in ((wq_s, bq_s, qT), (wk_s, bk_s, kT)):
                    for m in range(2):
                        for nb in range(NB):
                            ps = pj.tile([P, 512], FP32, tag="pj")
                            for h in range(HC):
                                nc.tensor.matmul(
                                    ps,
                                    lhsT=w_s[:, h, ts(m, P)],
                                    rhs=xTs[:, h, ts(nb, 512)],
                                    start=(h == 0),
                                    stop=False,
                                )
                            nc.tensor.matmul(
                                ps,
                                lhsT=b_s[:, ts(m, P)],
                                rhs=ones_s[:, 0:512],
                                start=False,
                                stop=True,
                            )
                            nc.vector.tensor_copy(out=dst[:, m, ts(nb, 512)], in_=ps)

                # v: [S, 256] = x @ Wv_g^T (+ bias), bf16
                for sc in range(SC):
                    ps = pj.tile([P, 512], FP32, tag="pj")
                    pv = ps[:, :C]
                    for h in range(HC):
                        nc.tensor.matmul(
                            pv,
                            lhsT=xTs[:, h, ts(sc, P)],
                            rhs=wv_s[:, h, :],
                            start=(h == 0),
                            stop=False,
                        )
                    nc.tensor.matmul(
                        pv,
                        lhsT=ones_s[:, 0:P],
                        rhs=bv_s[:],
                        start=False,
                        stop=True,
                    )
                    nc.vector.tensor_copy(out=vv[:, sc, :], in_=pv)

            # ============ attention, per qs super-block ============
            with (
                # sT/fbcast psum tiles in one 2-slot tag (4 banks);
                # ctxT accumulators, P4 out tiles and max-transposes in
                # another (4 banks).
                tc.tile_pool(name="stp", bufs=2, space="PSUM") as stp,
                tc.tile_pool(name="accp", bufs=1, space="PSUM") as accp,
                tc.tile_pool(name="tpp", bufs=2, space="PSUM") as tpp,
                tc.tile_pool(name="pu_pool", bufs=32) as pu_pool,
                tc.tile_pool(name="fb_pool", bufs=2) as fb_pool,
                tc.tile_pool(name="osb_pool", bufs=3) as osb_pool,
                tc.tile_pool(name="frp_pool", bufs=2) as frp_pool,
            ):
                for Q in range(NQ):
                    for p in range(2):
                        # ---- P2: sT scores (transposed) -> pu = exp ----
                        pu_tiles = [[None] * SC, [None] * SC]
                        for c in range(SC):
                            for l in range(2):
                                rows = slice(64 * l, 64 * l + 64)
                                st = stp.tile([P, QW], FP32, tag="B")
                                for u in range(2):
                                    nc.tensor.matmul(
                                        st[:, ts(u, 512)],
                                        lhsT=kT[rows, p, ts(c, P)],
                                        rhs=qT[rows, p, ds(Q * QW + u * 512, 512)],
                                        start=True,
                                        stop=True,
                                    )
                                pu = pu_pool.tile([P, QW], BF16, tag="pu")
                                nc.scalar.activation(
                                    out=pu,
                                    in_=st,
                                    func=mybir.ActivationFunctionType.Exp,
                                    bias=mb_s[:, c : c + 1],
                                    scale=0.125,
                                )
                                pu_tiles[l][c] = pu

                        # ---- P3 part 1: PV matmuls into ctx psum ----
                        cx = accp.tile([P, QW], FP32, tag="C")
                        for c in range(SC):
                            for l in range(2):
                                for u in range(2):
                                    nc.tensor.matmul(
                                        cx[ds(64 * l, 64), ts(u, 512)],
                                        lhsT=vv[:, c, ds(128 * p + 64 * l, 64)],
                                        rhs=pu_tiles[l][c][:, ts(u, 512)],
                                        start=(c == 0),
                                        stop=(c == SC - 1),
                                    )

                        # ---- rowmax(pu): in-place chunk-pair max tree over
                        # the pu tiles (safe: PV has consumed them), then a
                        # PE transpose per query block + free-dim reduce ----
                        for l in range(2):
                            stride = 1
                            while stride < SC:
                                for i in range(0, SC, 2 * stride):
                                    nc.vector.tensor_tensor(
                                        out=pu_tiles[l][i][:],
                                        in0=pu_tiles[l][i][:],
                                        in1=pu_tiles[l][i + stride][:],
                                        op=mybir.AluOpType.max,
                                    )
                                stride *= 2
                            R = pu_tiles[l][0]
                            for b8 in range(8):
                                mtp = tpp.tile([P, P], BF16, tag="T")
                                nc.tensor.transpose(mtp, R[:, ts(b8, P)], ident_bf)
                                nc.vector.reduce_max(
                                    out=mcols[:, p, Q * 8 + b8, l : l + 1],
                                    in_=mtp,
                                    axis=mybir.AxisListType.X,
                                )

                        # ---- frTp = 1/max(pu) transposed to qs-free ----
                        mt = tpp.tile([16, P], FP32, tag="T")
                        nc.tensor.transpose(
                            mt,
                            mcols[:, p, ds(Q * 8, 8), :].rearrange("p a b -> p (a b)"),
                            ident,
                        )
                        frTp = frp_pool.tile([16, P], FP32, tag="fr")
                        nc.vector.reciprocal(out=frTp, in_=mt)

                        # ---- fbcast: broadcast frTp to [128, QW] columns ----
                        fb_ps = stp.tile([P, QW], FP32, tag="B")
                        for qbl in range(8):
                            nc.tensor.matmul(
                                fb_ps[:, ts(qbl, P)],
                                lhsT=sel16[:, qbl, :],
                                rhs=frTp[:],
                                start=True,
                                stop=True,
                            )
                        fb_sb = fb_pool.tile([P, QW], FP32, tag="fb")
                        nc.vector.tensor_copy(out=fb_sb, in_=fb_ps)

                        # ---- P3 part 2: rescale ctx by 1/max and store ----
                        nc.vector.tensor_tensor(
                            out=ctxT[:, p, ds(Q * QW, QW)],
                            in0=cx[:],
                            in1=fb_sb[:],
                            op=mybir.AluOpType.mult,
                        )


                    # ---- P4: output projection for this Q ----
                    for qb in range(Q * 8, Q * 8 + 8):
                        op_ps = accp.tile([P, 1024], FP32, tag="C")
                        for ob in range(2):
                            for p in range(2):
                                nc.tensor.matmul(
                                    op_ps[:, ts(ob, 512)],
                                    lhsT=ctxT[:, p, ts(qb, P)],
                                    rhs=wo_s[:, p, ds(ob * 512, 512)],
                                    start=(p == 0),
                                    stop=(p == 1),
                                )
                        o_sb = osb_pool.tile([P, 1024], FP32, tag="osb")
                        nc.vector.tensor_copy(out=o_sb, in_=op_ps)
                        nc.sync.dma_start(out_d[ts(qb, P), :], o_sb)

    nc.compile()
    return nc


def _sel_const():
    sel = np.zeros((16, 8, P), dtype=np.float32)
    for qbl in range(8):
        sel[2 * qbl, qbl, 0:64] = 1.0
        sel[2 * qbl + 1, qbl, 64:128] = 1.0
    return sel


def _prep_inputs(hidden_states, attention_mask, Wq, bq, Wk, bk, Wv, bv,
                 Wo, bo, beta, gamma):
    g_scalar = float(np.asarray(gamma).reshape(-1)[0])
    bf = ml_dtypes.bfloat16
    in_maps = []
    for core in range(NCORES):
        b, g = core // NGROUPS, core % NGROUPS
        sl = slice(g * C, (g + 1) * C)
        mb = ((1.0 - np.asarray(attention_mask)[b]) * -10000.0).astype(np.float32)
        in_maps.append({
            "xT": np.ascontiguousarray(np.asarray(hidden_states)[b].T).astype(bf),
            "wqT": np.ascontiguousarray(np.asarray(Wq)[sl, :].T).astype(bf),
            "wkT": np.ascontiguousarray(np.asarray(Wk)[sl, :].T).astype(bf),
            "wvT": np.ascontiguousarray(np.asarray(Wv)[sl, :].T).astype(bf),
            "woT": (np.ascontiguousarray(np.asarray(Wo)[:, sl].T)
                    / g_scalar).astype(bf),
            "bq": np.asarray(bq)[sl].reshape(1, C).astype(bf),
            "bk": np.asarray(bk)[sl].reshape(1, C).astype(bf),
            "bv": np.asarray(bv)[sl].reshape(1, C).astype(bf),
            "mb": np.ascontiguousarray(mb.reshape(S // P, P).T),
            "sel": _sel_const(),
        })
    return in_maps


def kernel(**inputs):
    global _cached, _last_results
    if _cached is None:
        _cached = _build_program()
    nc = _cached
    in_maps = _prep_inputs(**inputs)
    os.environ["BASS_NEVER_TRACE"] = "1"  # no NTFF hook on this axon client
    res = run_bass_kernel_spmd(nc, in_maps, core_ids=list(range(NCORES)))
    _last_results = res
    bo = np.asarray(inputs["bo"], dtype=np.float32)
    out = np.zeros((B, S, HID), dtype=np.float32)
    for core in range(NCORES):
        out[core // NGROUPS] += res.results[core]["outp"]
    out += bo[None, None, :]
    return out
